# revision 43
# baseline (speedup 1.0000x reference)
"""Trainium2 Bass kernel for Transformer-XL style relative-position MHA.

Problem shapes (hardcoded): B=8, SEG=512, MEM=512, MODEL=1024, H=16, D=64.
Sharding: pure data-parallel over batch -> core b computes batch element b.

Head split quirk (torch .view flat reinterpret): for a [T, H*D] projection P,
head h's matrix is P_h[m, d] = P[64h + m//16, 64*(m%16) + d] (T=1024; for the
[512, H*D] q projection the row term is 32h + i//16).

Per-head layouts ("lay": [d (+ u-row), h*T + m]) are built with ACT/DVE engine
copies straight out of the projection PSUM chunks: psum chunk dt holds output
channels hd in [128dt, 128dt+128) = 64b+d for b in {2dt, 2dt+1}; partition
half eps gives b = 2dt+eps, so two strided engine copies per chunk land the
data in lay order. (Engines handle arbitrary-stride APs at full rate; DMA
descriptor scatter for the same transform ran at ~1 GB/s.)

The r projection (R[-T:] @ w_r) and its lay layout + u2.r row are batch
independent and precomputed on the host (rlay input).

Scores: ac = (q + ones*u1.k-row) matmuls as before; bd is computed RAW
(no exp) into a [S, H, T] bf16 DRAM buffer and read back with the skewed
(circulant) access pattern; masking is additive (-30000 on the triangular
corner, which exactly covers the out-of-range spill), then ONE
exp((ac+bd)/8) with accum_out row sums. Softmax normalization (x att_mask)
is folded into the p@v PSUM evacuation via a PE-transposed sums row and a
partition-broadcast multiply.
"""

import functools
import sys

import numpy as np

sys.path.insert(0, "/opt/trn_rl_repo")

import ml_dtypes  # noqa: E402

import concourse.bass as bass  # noqa: E402
import concourse.mybir as mybir  # noqa: E402
import concourse.tile as tile  # noqa: E402

B, SEG, MEM, MODEL, H, D = 8, 512, 512, 1024, 16, 64
TOT = SEG + MEM
HD = H * D
NCORES = 8
IT = SEG // 128                # 4 row tiles of 128 queries
JMAX = [640, 768, 896, 1024]   # per row-tile: columns beyond are fully masked
MMIN = [384, 256, 128, 0]      # per row-tile: smallest rel index m read
NEG = -30000.0                 # additive mask (exp(NEG/8) == 0 in fp)

F32 = mybir.dt.float32
BF16 = mybir.dt.bfloat16
AF = mybir.ActivationFunctionType
OP = mybir.AluOpType

bf16_np = ml_dtypes.bfloat16


def _emit(tc, t):
    nc = tc.nc
    ctxs = []

    def pool(name, bufs, space="SBUF"):
        p = tc.tile_pool(name=name, bufs=bufs, space=space)
        ctxs.append(p)
        return p.__enter__()

    csts = pool("csts", 1)
    lay32 = pool("lay32", 2)      # rlay2 + klay2 [128, 8192] bf16 (16KB/part each)
    qlp = pool("qlp", 2)          # qlayU1 / qlayU2 [128, 4096] bf16
    qbp = pool("qbp", 1)          # qbase [128,4096] then attTh [64,8192] bf16
    vtgp = pool("vtgp", 2)        # vtg group [64, 4*TOT] bf16
    vlp = pool("vlp", 8)          # vlay [128, 16*64] bf16 x 8 j-tiles
    hts_p = pool("htsp", 8)       # hT bf16 [128, TOT]
    wpool = pool("wpool", 8)      # streaming weights [128,1024] bf16
    xp = pool("xp", 2)            # x residual f32 [128, 1024]
    ebgp = pool("ebgp", 2)        # bd pair evac [128, 2*1024] bf16
    ebsp = pool("ebsp", 3)        # skewed pair read [128, 2*1024] bf16
    workp = pool("workp", 3)      # s / pexp tiles [128, 1024] bf16
    ptsp = pool("ptsp", 2)        # pT [128, 1024] bf16
    attp = pool("attp", 8)        # attT [128,512] bf16
    yp = pool("yp", 2)            # mlp y/o f32
    small = pool("small", 3)
    ps = pool("ps", 2, space="PSUM")      # [128,1024] f32 (2 banks)
    pst = pool("pst", 2, space="PSUM")    # transposes [128,128] / [1,512]
    psv = pool("psv", 2, space="PSUM")    # pv accum [64,128] f32

    # ---- constants ----
    ident = csts.tile([128, 128], BF16, tag="ident")
    nc.scalar.dma_start(ident, t["idm"][:, :])
    triB = csts.tile([128, 128], BF16, tag="triB")
    nc.scalar.dma_start(triB, t["trim"][:, :])
    u1p = csts.tile([128, 8], F32, tag="u1p")
    nc.scalar.dma_start(u1p, t["u1p"][:, :])
    u2p = csts.tile([128, 8], F32, tag="u2p")
    nc.scalar.dma_start(u2p, t["u2p"][:, :])
    masks = csts.tile([128, 4], F32, tag="masks")
    nc.scalar.dma_start(masks, t["maskc"][:, :])
    epsb = csts.tile([128, 1], F32, tag="epsb")
    nc.vector.memset(epsb, 1e-5)
    gam = csts.tile([128, MODEL], BF16, tag="gam")
    nc.gpsimd.dma_start(gam, bass.AP(tensor=t["gam"], offset=0, ap=[[0, 128], [1, MODEL]]))
    bet = csts.tile([128, MODEL], BF16, tag="bet")
    nc.gpsimd.dma_start(bet, bass.AP(tensor=t["bet"], offset=0, ap=[[0, 128], [1, MODEL]]))

    ebuf = t["ebuf"]

    # ---- zero strip: rows [0,384) x all heads x m in [0,128) of ebuf ----
    zs = csts.tile([128, 4 * 128], BF16, tag="zs")
    nc.vector.memset(zs, 0.0)
    for blk in range(3):
        for hb in range(4):
            dst = bass.AP(
                tensor=ebuf,
                offset=blk * 128 * H * TOT + hb * 4 * TOT,
                ap=[[H * TOT, 128], [TOT, 4], [1, 128]],
            )
            nc.gpsimd.dma_start(dst, zs.rearrange("p (h m) -> p h m", h=4))

    # ---- rlay2 (host precomputed, head-pair stacked) ----
    rlay = lay32.tile([128, 8 * TOT], BF16, tag="lay32", name="rlay")
    nc.scalar.dma_start(rlay, t["rlay"][:, :])

    # ---- load hT (bf16) ----
    hts = []
    for mt in range(8):
        ht = hts_p.tile([128, TOT], BF16, tag="ht", name=f"ht{mt}")
        eng = nc.sync if mt % 2 == 0 else nc.scalar
        eng.dma_start(ht, t["hT"][mt * 128:(mt + 1) * 128, :])
        hts.append(ht)

    def stream_w(key, eng_sel=0):
        ws = []
        for mt in range(8):
            w = wpool.tile([128, HD], BF16, tag="w", name=f"{key}{mt}")
            eng = nc.sync if (mt + eng_sel) % 2 == 0 else nc.scalar
            eng.dma_start(w, t[key][mt * 128:(mt + 1) * 128, :])
            ws.append(w)
        return ws

    # ---- q projection -> qbase (head-pair stacked) via engine copies ----
    # qbase[64*pi + d, g*512 + 16a + b] = Qpsum[b-chunk][64(b%2)+d, 64g+32pi+a]
    qbase = qbp.tile([128, 8 * SEG], BF16, tag="qb")
    wqs = stream_w("wq", 0)
    for dt in range(8):
        acc = ps.tile([128, SEG], F32, tag="mm", name=f"qmm{dt}")
        for mt in range(8):
            nc.tensor.matmul(
                acc,
                lhsT=wqs[mt][:, dt * 128:(dt + 1) * 128],
                rhs=hts[mt][:, SEG:],
                start=(mt == 0),
                stop=(mt == 7),
            )
        qstep = qbase[:, :].ap[0][0]
        astep = acc[:, :].ap[0][0]
        for eps in range(2):
            b_ = 2 * dt + eps
            for pi in range(2):
                src = bass.AP(
                    tensor=acc[:, :].tensor,
                    offset=acc[:, :].offset + eps * 64 * astep + 32 * pi,
                    ap=[[astep, 64], [64, 8], [1, 32]],
                )
                dst = bass.AP(
                    tensor=qbase[:, :].tensor,
                    offset=qbase[:, :].offset + 64 * pi * qstep + b_,
                    ap=[[qstep, 64], [512, 8], [16, 32]],
                )
                if (eps + pi) % 2 == 0:
                    nc.scalar.copy(dst, src)
                else:
                    nc.vector.tensor_copy(dst, src)
    # u-folded q variants: qlayU1 (for ac vs k), qlayU2 (for bd vs r)
    qlayU1 = qlp.tile([128, 8 * SEG], BF16, tag="qlay", name="qlayU1")
    qlayU2 = qlp.tile([128, 8 * SEG], BF16, tag="qlay", name="qlayU2")
    for g in range(8):
        sl = slice(g * SEG, (g + 1) * SEG)
        nc.vector.tensor_scalar_add(qlayU1[:, sl], qbase[:, sl], u1p[:, g:g + 1])
        nc.vector.tensor_scalar_add(qlayU2[:, sl], qbase[:, sl], u2p[:, g:g + 1])

    # ---- bd production (raw, row-packed head pairs) -> ebuf ----
    for g in range(8):
        for it in range(IT):
            m0, w_ = MMIN[it], TOT - MMIN[it]
            ebg = ebgp.tile([128, 2 * w_], BF16, tag="ebg", name=f"ebg{g}_{it}")
            for e in range(2):
                bd = ps.tile([128, w_], F32, tag="mm", name=f"bd{2 * g + e}_{it}")
                pb = 64 * e
                c0 = 0
                while c0 < w_:
                    cw = min(512, w_ - c0)
                    nc.tensor.matmul(
                        bd[:, c0:c0 + cw],
                        lhsT=qlayU2[pb:pb + 64,
                                    g * SEG + it * 128:g * SEG + (it + 1) * 128],
                        rhs=rlay[pb:pb + 64,
                                 g * TOT + m0 + c0:g * TOT + m0 + c0 + cw],
                        start=True,
                        stop=True,
                    )
                    c0 += cw
                if e == 0:
                    nc.scalar.copy(ebg[:, e * w_:(e + 1) * w_], bd)
                else:
                    nc.vector.tensor_copy(ebg[:, e * w_:(e + 1) * w_], bd)
            dst = bass.AP(
                tensor=ebuf,
                offset=it * 128 * H * TOT + (2 * g) * TOT + m0,
                ap=[[H * TOT, 128], [TOT, 2], [1, w_]],
            )
            nc.sync.dma_start(dst, ebg.rearrange("p (e w) -> p e w", e=2))

    # ---- k projection -> klay2 (head-pair stacked) ----
    # klay2[64*pi + d, g*1024 + 16a + b] = Kpsum[b-chunk][64(b%2)+d, 128g+64pi+a]
    klay = lay32.tile([128, 8 * TOT], BF16, tag="lay32", name="klay")
    kst = klay[:, :].ap[0][0]
    koff = klay[:, :].offset
    wks = stream_w("wk", 1)
    for dt in range(8):
        acc = ps.tile([128, TOT], F32, tag="mm", name=f"kmm{dt}")
        for c0 in range(0, TOT, 512):
            for mt in range(8):
                nc.tensor.matmul(
                    acc[:, c0:c0 + 512],
                    lhsT=wks[mt][:, dt * 128:(dt + 1) * 128],
                    rhs=hts[mt][:, c0:c0 + 512],
                    start=(mt == 0),
                    stop=(mt == 7),
                )
        astep = acc[:, :].ap[0][0]
        for eps in range(2):
            b_ = 2 * dt + eps
            for pi in range(2):
                src = bass.AP(
                    tensor=acc[:, :].tensor,
                    offset=acc[:, :].offset + eps * 64 * astep + 64 * pi,
                    ap=[[astep, 64], [128, 8], [1, 64]],
                )
                dst = bass.AP(
                    tensor=klay[:, :].tensor,
                    offset=koff + 64 * pi * kst + 64 * b_,
                    ap=[[kst, 64], [1024, 8], [1, 64]],
                )
                if (eps + pi) % 2 == 0:
                    nc.scalar.copy(dst, src)
                else:
                    nc.vector.tensor_copy(dst, src)

    # ---- v projection per 4-head group -> vtg -> PE-transpose -> vls ----
    vls = [
        vlp.tile([128, H * 64], BF16, tag="vl", name=f"vl{jb}") for jb in range(8)
    ]
    wvs = stream_w("wv", 0)
    for half in range(2):
        vtgs = [
            vtgp.tile([64, 4 * TOT], BF16, tag="vtg", name=f"vtg{2 * half + gg}")
            for gg in range(2)
        ]
        for dt in range(8):
            acc = ps.tile([128, 512], F32, tag="mm", name=f"vmm{half}_{dt}")
            for mt in range(8):
                nc.tensor.matmul(
                    acc,
                    lhsT=wvs[mt][:, dt * 128:(dt + 1) * 128],
                    rhs=hts[mt][:, 512 * half:512 * (half + 1)],
                    start=(mt == 0),
                    stop=(mt == 7),
                )
            astep = acc[:, :].ap[0][0]
            for eps in range(2):
                b_ = 2 * dt + eps
                for gg in range(2):
                    vtg = vtgs[gg]
                    src = bass.AP(
                        tensor=acc[:, :].tensor,
                        offset=acc[:, :].offset + eps * 64 * astep + 256 * gg,
                        ap=[[astep, 64], [64, 4], [1, 64]],
                    )
                    dst = bass.AP(
                        tensor=vtg[:, :].tensor,
                        offset=vtg[:, :].offset + b_,
                        ap=[[vtg[:, :].ap[0][0], 64], [1024, 4], [16, 64]],
                    )
                    if (dt + eps + gg) % 2 == 0:
                        nc.scalar.copy(dst, src)
                    else:
                        nc.vector.tensor_copy(dst, src)
        for gg in range(2):
            g = 2 * half + gg
            vtg = vtgs[gg]
            for hh in range(4):
                h = 4 * g + hh
                for jb in range(8):
                    tp = pst.tile([128, 64], BF16, tag="tp", name=f"vt{h}_{jb}")
                    nc.tensor.transpose(
                        tp, vtg[0:64, hh * TOT + jb * 128:hh * TOT + (jb + 1) * 128],
                        ident[0:64, 0:64],
                    )
                    nc.scalar.copy(vls[jb][:, h * 64:(h + 1) * 64], tp)

    # ---- scores / softmax / p@v ----
    attTh = qbp.tile([64, H * SEG], BF16, tag="qb", name="attTh")
    for it in range(IT):
        jm = JMAX[it]
        nblk = jm // 128
        i0 = it * 128
        for hp in range(8):
            ebs = ebsp.tile([128, 2 * jm], BF16, tag="ebs", name=f"ebs{it}_{hp}")
            src = bass.AP(
                tensor=ebuf,
                offset=i0 * H * TOT + (2 * hp) * TOT + (511 - i0),
                ap=[[H * TOT - 1, 128], [TOT, 2], [1, jm]],
            )
            nc.scalar.dma_start(ebs.rearrange("p (e w) -> p e w", e=2), src)
            pts_pair = []
            for e in range(2):
                h = 2 * hp + e
                # additive triangular corner mask (covers the circulant spill)
                nc.gpsimd.tensor_add(
                    ebs[:, e * jm + jm - 128:(e + 1) * jm],
                    ebs[:, e * jm + jm - 128:(e + 1) * jm],
                    triB,
                )
                acps = ps.tile([128, jm], F32, tag="mm", name=f"ac{it}_{h}")
                pb = 64 * e
                kst_ = klay[:, :].ap[0][0]
                c0 = 0
                while c0 < jm:
                    cw = min(512, jm - c0)
                    rhs = bass.AP(
                        tensor=klay[:, :].tensor,
                        offset=klay[:, :].offset + pb * kst_ + hp * TOT + c0 // 16,
                        ap=[[kst_, 64], [1, cw // 16], [64, 16]],
                    )
                    nc.tensor.matmul(
                        acps[:, c0:c0 + cw],
                        lhsT=qlayU1[pb:pb + 64,
                                    hp * SEG + i0:hp * SEG + i0 + 128],
                        rhs=rhs,
                        start=True,
                        stop=False,
                    )
                    # accumulate shifted-bd scores: acps += I.T @ ebs
                    nc.tensor.matmul(
                        acps[:, c0:c0 + cw],
                        lhsT=ident,
                        rhs=ebs[:, e * jm + c0:e * jm + c0 + cw],
                        start=False,
                        stop=True,
                    )
                    c0 += cw
                pexp = workp.tile([128, jm], BF16, tag="wk", name=f"p{it}_{h}")
                sums = small.tile([128, 1], F32, tag="sums", name=f"sm{it}_{h}")
                nc.scalar.activation(pexp, acps, AF.Exp, scale=0.125, accum_out=sums)
                rec = small.tile([128, 1], F32, tag="rec", name=f"rc{it}_{h}")
                nc.vector.reciprocal(rec, sums)
                alph = small.tile([128, 1], F32, tag="alph", name=f"al{it}_{h}")
                nc.vector.tensor_mul(alph, rec, masks[:, it:it + 1])
                nc.vector.tensor_scalar_mul(pexp, pexp, alph)
                pts = ptsp.tile([128, jm], BF16, tag="pts", name=f"pt{it}_{h}")
                for jb in range(nblk):
                    tp = pst.tile([128, 128], BF16, tag="tp", name=f"tp{it}_{h}_{jb}")
                    nc.tensor.transpose(tp, pexp[:, jb * 128:(jb + 1) * 128], ident)
                    if jb % 2 == 0:
                        nc.vector.tensor_copy(pts[:, jb * 128:(jb + 1) * 128], tp)
                    else:
                        nc.scalar.copy(pts[:, jb * 128:(jb + 1) * 128], tp)
                pts_pair.append(pts)
            # packed p@v: two heads as PE column-tiles into one PSUM tile
            pv = psv.tile([128, 128], F32, tag="pv", name=f"pv{it}_{hp}")
            for jb in range(nblk):
                for e in range(2):
                    h = 2 * hp + e
                    nc.tensor.matmul(
                        pv[64 * e:64 * (e + 1), :],
                        lhsT=vls[jb][:, 64 * h:64 * h + 64],
                        rhs=pts_pair[e][:, jb * 128:(jb + 1) * 128],
                        start=(jb == 0),
                        stop=(jb == nblk - 1),
                    )
            for e in range(2):
                h = 2 * hp + e
                nc.scalar.copy(
                    attTh[:, h * SEG + i0:h * SEG + i0 + 128],
                    pv[64 * e:64 * (e + 1), :],
                )

    # ---- att DRAM hop: attP[dd, cc*512+32h+rr] = attTh[dd, h*512+16rr+cc] ----
    attP = vtgp.tile([64, H * SEG], BF16, tag="vtg", name="attP")
    ao = attTh[:, :].offset
    astep = attTh[:, :].ap[0][0]
    src = bass.AP(
        tensor=attTh[:, :].tensor, offset=ao,
        ap=[[astep, 64], [1, 16], [512, 16], [16, 32]],
    )
    po = attP[:, :].offset
    pstep = attP[:, :].ap[0][0]
    dst = bass.AP(
        tensor=attP[:, :].tensor, offset=po,
        ap=[[pstep, 64], [512, 16], [32, 16], [1, 32]],
    )
    nc.vector.tensor_copy(dst, src)
    nc.sync.dma_start(
        bass.AP(tensor=t["attd"], offset=0, ap=[[H * SEG, 64], [1, H * SEG]]),
        attP,
    )
    atts = []
    for a in range(8):
        at = attp.tile([128, SEG], BF16, tag="att", name=f"att{a}")
        for ccp in range(2):
            src = bass.AP(
                tensor=t["attd"],
                offset=(2 * a + ccp) * 512,
                ap=[[H * SEG, 64], [1, 512]],
            )
            eng = nc.scalar if (a + ccp) % 2 == 0 else nc.sync
            eng.dma_start(at[ccp * 64:(ccp + 1) * 64, :], src)
        atts.append(at)

    # ---- mlp + residual + layernorm ----
    mlps = stream_w("mlpw", 1)
    xs = []
    for it in range(IT):
        x = xp.tile([128, MODEL], F32, tag="x", name=f"x{it}")
        eng = nc.sync if it % 2 == 0 else nc.scalar
        eng.dma_start(x, t["x_sm"][it * 128:(it + 1) * 128, :])
        xs.append(x)
    for it in range(IT):
        acc = ps.tile([128, MODEL], F32, tag="mm", name=f"mlp{it}")
        for half in range(2):
            for dt in range(8):
                nc.tensor.matmul(
                    acc[:, half * 512:(half + 1) * 512],
                    lhsT=atts[dt][:, it * 128:(it + 1) * 128],
                    rhs=mlps[dt][:, half * 512:(half + 1) * 512],
                    start=(dt == 0),
                    stop=(dt == 7),
                )
        y = yp.tile([128, MODEL], F32, tag="y", name=f"y{it}", bufs=1)
        ysum = small.tile([128, 1], F32, tag="ysum", name=f"ys{it}")
        nc.vector.scalar_tensor_tensor(
            out=y, in0=acc, scalar=1.0, in1=xs[it],
            op0=OP.mult, op1=OP.add, accum_out=ysum,
        )
        sq = ps.tile([128, MODEL], F32, tag="mm", name=f"sq{it}")
        ysq = small.tile([128, 1], F32, tag="ysq", name=f"yq{it}")
        nc.scalar.activation(sq, y, AF.Square, accum_out=ysq)
        mu = small.tile([128, 1], F32, tag="mu", name=f"mu{it}")
        nc.scalar.mul(mu, ysum, 1.0 / MODEL)
        msq = small.tile([128, 1], F32, tag="msq", name=f"mq{it}")
        nc.scalar.mul(msq, ysq, 1.0 / MODEL)
        mu2 = small.tile([128, 1], F32, tag="mu2", name=f"m2{it}")
        nc.vector.tensor_mul(mu2, mu, mu)
        var = small.tile([128, 1], F32, tag="var", name=f"va{it}")
        nc.vector.tensor_tensor(out=var, in0=msq, in1=mu2, op=OP.subtract)
        std = small.tile([128, 1], F32, tag="std", name=f"sd{it}")
        nc.scalar.activation(std, var, AF.Sqrt, bias=epsb)
        rstd = small.tile([128, 1], F32, tag="rstd", name=f"rs{it}")
        nc.vector.reciprocal(rstd, std)
        o = yp.tile([128, MODEL], F32, tag="o", name=f"o{it}", bufs=1)
        nc.vector.tensor_scalar(
            out=o, in0=y, scalar1=mu, scalar2=rstd,
            op0=OP.subtract, op1=OP.mult,
        )
        nc.vector.tensor_mul(o, o, gam)
        nc.vector.tensor_add(o, o, bet)
        nc.sync.dma_start(t["yout"][it * 128:(it + 1) * 128, :], o)

    for p_ in reversed(ctxs):
        p_.__exit__(None, None, None)


def _split_ctrl_waits(nc, maxw=1):
    """The container's walrus rejects instructions carrying more than 2 sem
    waits ("Too many sync wait commands"). Move excess waits onto preceding
    same-engine NoOps (engines execute their stream in order, so the waits
    still complete before the instruction issues)."""
    n = 0
    for bb in nc.main_func.blocks:
        changed = False
        out = []
        for ins in bb.instructions:
            lim = maxw
            si = ins.sync_info
            if si is not None and si.on_wait and len(si.on_wait) > lim:
                waits = list(si.on_wait)
                while len(waits) > lim:
                    chunk, waits = waits[:lim], waits[lim:]
                    nop = mybir.InstNoOp(
                        name=f"I-wsplit{n}",
                        engine=ins.engine,
                        sync_info=mybir.SyncInfo(on_wait=list(chunk), on_update=[]),
                    )
                    n += 1
                    out.append(nop)
                si.on_wait = waits
                changed = True
            out.append(ins)
        if changed:
            bb.instructions = out


@functools.lru_cache(maxsize=1)
def _build():
    nc = bass.Bass()
    t = {}
    t["hT"] = nc.dram_tensor("hT", [MODEL, TOT], BF16, kind="ExternalInput")
    t["x_sm"] = nc.dram_tensor("x_sm", [SEG, MODEL], F32, kind="ExternalInput")
    for w in ("wq", "wk", "wv"):
        t[w] = nc.dram_tensor(w, [MODEL, HD], BF16, kind="ExternalInput")
    t["mlpw"] = nc.dram_tensor("mlpw", [HD, MODEL], BF16, kind="ExternalInput")
    t["rlay"] = nc.dram_tensor("rlay", [128, 8 * TOT], BF16, kind="ExternalInput")
    t["u1p"] = nc.dram_tensor("u1p", [128, 8], F32, kind="ExternalInput")
    t["u2p"] = nc.dram_tensor("u2p", [128, 8], F32, kind="ExternalInput")
    t["maskc"] = nc.dram_tensor("maskc", [128, 4], F32, kind="ExternalInput")
    t["gam"] = nc.dram_tensor("gam", [1, MODEL], BF16, kind="ExternalInput")
    t["bet"] = nc.dram_tensor("bet", [1, MODEL], BF16, kind="ExternalInput")
    t["trim"] = nc.dram_tensor("trim", [128, 128], BF16, kind="ExternalInput")
    t["idm"] = nc.dram_tensor("idm", [128, 128], BF16, kind="ExternalInput")
    t["ebuf"] = nc.dram_tensor("ebuf", [SEG, H, TOT], BF16)
    t["attd"] = nc.dram_tensor("attd", [64, H * SEG], BF16)
    t["yout"] = nc.dram_tensor("yout", [SEG, MODEL], F32, kind="ExternalOutput")

    with tile.TileContext(nc) as tc:
        _emit(tc, t)
    _split_ctrl_waits(nc)
    return nc


def _host_inputs(inputs):
    x = np.asarray(inputs["x"], np.float32)
    mem = np.asarray(inputs["mem"], np.float32)
    att_mask = np.asarray(inputs["att_mask"], np.float32)
    u1 = np.asarray(inputs["u1"], np.float32).reshape(H, D)
    u2 = np.asarray(inputs["u2"], np.float32).reshape(H, D)
    R = np.asarray(inputs["R"], np.float32)

    h = np.concatenate([mem, x], axis=1)  # [B, TOT, MODEL]

    # host r projection + head-pair-stacked lay layout
    RW = R[-TOT:] @ np.asarray(inputs["w_r"], np.float32)           # [TOT, HD]
    rl64 = RW.reshape(16, 64, 16, 64).transpose(3, 0, 1, 2).reshape(64, H, TOT)
    rlay = np.zeros((128, 8 * TOT), np.float32)
    rlay[0:64] = rl64[:, 0::2].reshape(64, 8 * TOT)
    rlay[64:128] = rl64[:, 1::2].reshape(64, 8 * TOT)
    u1p = np.zeros((128, 8), np.float32)
    u1p[0:64] = u1.T[:, 0::2]
    u1p[64:128] = u1.T[:, 1::2]
    u2p = np.zeros((128, 8), np.float32)
    u2p[0:64] = u2.T[:, 0::2]
    u2p[64:128] = u2.T[:, 1::2]

    trim = np.where(
        np.tril(np.ones((128, 128), np.float32)) > 0, 0.0, NEG
    ).astype(bf16_np)

    shared = {
        "wq": np.asarray(inputs["w_q"], np.float32).astype(bf16_np),
        "wk": np.asarray(inputs["w_k"], np.float32).astype(bf16_np),
        "wv": np.asarray(inputs["w_v"], np.float32).astype(bf16_np),
        "mlpw": np.asarray(inputs["mlp_w"], np.float32).astype(bf16_np),
        "rlay": rlay.astype(bf16_np),
        "u1p": u1p,
        "u2p": u2p,
        "gam": np.asarray(inputs["ln_gamma"], np.float32).reshape(1, MODEL).astype(bf16_np),
        "bet": np.asarray(inputs["ln_beta"], np.float32).reshape(1, MODEL).astype(bf16_np),
        "trim": trim,
        "idm": np.eye(128, dtype=np.float32).astype(bf16_np),
    }
    in_maps = []
    for b in range(NCORES):
        m = dict(shared)
        m["hT"] = np.ascontiguousarray(h[b].T).astype(bf16_np)
        m["x_sm"] = np.ascontiguousarray(x[b])
        m["maskc"] = np.ascontiguousarray(att_mask[b].reshape(4, 128).T)
        in_maps.append(m)
    return in_maps


def kernel(**inputs) -> np.ndarray:
    from concourse.bass_utils import run_bass_kernel_spmd

    nc = _build()
    in_maps = _host_inputs(inputs)
    res = run_bass_kernel_spmd(nc, in_maps, list(range(NCORES)))
    out = np.stack([np.asarray(res.results[b]["yout"]) for b in range(NCORES)])
    return out.astype(np.float32)


# revision 44
# speedup vs baseline: 1.0122x; 1.0122x over previous
"""Trainium2 Bass kernel for Transformer-XL style relative-position MHA.

Problem shapes (hardcoded): B=8, SEG=512, MEM=512, MODEL=1024, H=16, D=64.
Sharding: pure data-parallel over batch -> core b computes batch element b.

Head split quirk (torch .view flat reinterpret): for a [T, H*D] projection P,
head h's matrix is P_h[m, d] = P[64h + m//16, 64*(m%16) + d] (T=1024; for the
[512, H*D] q projection the row term is 32h + i//16).

Per-head layouts ("lay": [d (+ u-row), h*T + m]) are built with ACT/DVE engine
copies straight out of the projection PSUM chunks: psum chunk dt holds output
channels hd in [128dt, 128dt+128) = 64b+d for b in {2dt, 2dt+1}; partition
half eps gives b = 2dt+eps, so two strided engine copies per chunk land the
data in lay order. (Engines handle arbitrary-stride APs at full rate; DMA
descriptor scatter for the same transform ran at ~1 GB/s.)

The r projection (R[-T:] @ w_r) and its lay layout + u2.r row are batch
independent and precomputed on the host (rlay input).

Scores: ac = (q + ones*u1.k-row) matmuls as before; bd is computed RAW
(no exp) into a [S, H, T] bf16 DRAM buffer and read back with the skewed
(circulant) access pattern; masking is additive (-30000 on the triangular
corner, which exactly covers the out-of-range spill), then ONE
exp((ac+bd)/8) with accum_out row sums. Softmax normalization (x att_mask)
is folded into the p@v PSUM evacuation via a PE-transposed sums row and a
partition-broadcast multiply.
"""

import functools
import sys

import numpy as np

sys.path.insert(0, "/opt/trn_rl_repo")

import ml_dtypes  # noqa: E402

import concourse.bass as bass  # noqa: E402
import concourse.mybir as mybir  # noqa: E402
import concourse.tile as tile  # noqa: E402

B, SEG, MEM, MODEL, H, D = 8, 512, 512, 1024, 16, 64
TOT = SEG + MEM
HD = H * D
NCORES = 8
IT = SEG // 128                # 4 row tiles of 128 queries
JMAX = [640, 768, 896, 1024]   # per row-tile: columns beyond are fully masked
MMIN = [384, 256, 128, 0]      # per row-tile: smallest rel index m read
NEG = -30000.0                 # additive mask (exp(NEG/8) == 0 in fp)

F32 = mybir.dt.float32
BF16 = mybir.dt.bfloat16
AF = mybir.ActivationFunctionType
OP = mybir.AluOpType

bf16_np = ml_dtypes.bfloat16


def _emit(tc, t):
    nc = tc.nc
    ctxs = []

    def pool(name, bufs, space="SBUF"):
        p = tc.tile_pool(name=name, bufs=bufs, space=space)
        ctxs.append(p)
        return p.__enter__()

    csts = pool("csts", 1)
    lay32 = pool("lay32", 2)      # rlay2 + klay2 [128, 8192] bf16 (16KB/part each)
    qlp = pool("qlp", 2)          # qlayU1 / qlayU2 [128, 4096] bf16
    qbp = pool("qbp", 1)          # qbase [128,4096] then attTh [64,8192] bf16
    vtgp = pool("vtgp", 2)        # vtg group [64, 4*TOT] bf16
    vlp = pool("vlp", 8)          # vlay [128, 16*64] bf16 x 8 j-tiles
    hts_p = pool("htsp", 8)       # hT bf16 [128, TOT]
    wpool = pool("wpool", 8)      # streaming weights [128,1024] bf16
    xp = pool("xp", 2)            # x residual f32 [128, 1024]
    ebgp = pool("ebgp", 2)        # bd pair evac [128, 2*1024] bf16
    ebsp = pool("ebsp", 3)        # skewed pair read [128, 2*1024] bf16
    workp = pool("workp", 3)      # s / pexp tiles [128, 1024] bf16
    ptsp = pool("ptsp", 2)        # pT [128, 1024] bf16
    attp = pool("attp", 8)        # attT [128,512] bf16
    yp = pool("yp", 2)            # mlp y/o f32
    small = pool("small", 3)
    ps = pool("ps", 2, space="PSUM")      # [128,1024] f32 (2 banks)
    pst = pool("pst", 2, space="PSUM")    # transposes [128,128] / [1,512]
    psv = pool("psv", 2, space="PSUM")    # pv accum [64,128] f32

    # ---- constants ----
    ident = csts.tile([128, 128], BF16, tag="ident")
    nc.scalar.dma_start(ident, t["idm"][:, :])
    triB = csts.tile([128, 128], BF16, tag="triB")
    nc.scalar.dma_start(triB, t["trim"][:, :])
    u1p = csts.tile([128, 8], F32, tag="u1p")
    nc.scalar.dma_start(u1p, t["u1p"][:, :])
    u2p = csts.tile([128, 8], F32, tag="u2p")
    nc.scalar.dma_start(u2p, t["u2p"][:, :])
    masks = csts.tile([128, 4], F32, tag="masks")
    nc.scalar.dma_start(masks, t["maskc"][:, :])
    epsb = csts.tile([128, 1], F32, tag="epsb")
    nc.vector.memset(epsb, 1e-5)
    gam = csts.tile([128, MODEL], BF16, tag="gam")
    nc.gpsimd.dma_start(gam, bass.AP(tensor=t["gam"], offset=0, ap=[[0, 128], [1, MODEL]]))
    bet = csts.tile([128, MODEL], BF16, tag="bet")
    nc.gpsimd.dma_start(bet, bass.AP(tensor=t["bet"], offset=0, ap=[[0, 128], [1, MODEL]]))

    ebuf = t["ebuf"]

    # ---- zero strip: rows [0,384) x all heads x m in [0,128) of ebuf ----
    zs = csts.tile([128, 4 * 128], BF16, tag="zs")
    nc.vector.memset(zs, 0.0)
    for blk in range(3):
        for hb in range(4):
            dst = bass.AP(
                tensor=ebuf,
                offset=blk * 128 * H * TOT + hb * 4 * TOT,
                ap=[[H * TOT, 128], [TOT, 4], [1, 128]],
            )
            nc.gpsimd.dma_start(dst, zs.rearrange("p (h m) -> p h m", h=4))

    # ---- rlay2 (host precomputed, head-pair stacked) ----
    rlay = lay32.tile([128, 8 * TOT], BF16, tag="lay32", name="rlay")
    nc.scalar.dma_start(rlay, t["rlay"][:, :])

    # ---- load hT (bf16) ----
    hts = []
    for mt in range(8):
        ht = hts_p.tile([128, TOT], BF16, tag="ht", name=f"ht{mt}")
        eng = nc.sync if mt % 2 == 0 else nc.scalar
        eng.dma_start(ht, t["hT"][mt * 128:(mt + 1) * 128, :])
        hts.append(ht)

    def stream_w(key, eng_sel=0):
        ws = []
        for mt in range(8):
            w = wpool.tile([128, HD], BF16, tag="w", name=f"{key}{mt}")
            eng = nc.sync if (mt + eng_sel) % 2 == 0 else nc.scalar
            eng.dma_start(w, t[key][mt * 128:(mt + 1) * 128, :])
            ws.append(w)
        return ws

    # ---- q projection -> qbase (head-pair stacked) via engine copies ----
    # qbase[64*pi + d, g*512 + 16a + b] = Qpsum[b-chunk][64(b%2)+d, 64g+32pi+a]
    qbase = qbp.tile([128, 8 * SEG], BF16, tag="qb")
    wqs = stream_w("wq", 0)
    for dt in range(8):
        acc = ps.tile([128, SEG], F32, tag="mm", name=f"qmm{dt}")
        for mt in range(8):
            nc.tensor.matmul(
                acc,
                lhsT=wqs[mt][:, dt * 128:(dt + 1) * 128],
                rhs=hts[mt][:, SEG:],
                start=(mt == 0),
                stop=(mt == 7),
            )
        qstep = qbase[:, :].ap[0][0]
        astep = acc[:, :].ap[0][0]
        for eps in range(2):
            b_ = 2 * dt + eps
            for pi in range(2):
                src = bass.AP(
                    tensor=acc[:, :].tensor,
                    offset=acc[:, :].offset + eps * 64 * astep + 32 * pi,
                    ap=[[astep, 64], [64, 8], [1, 32]],
                )
                dst = bass.AP(
                    tensor=qbase[:, :].tensor,
                    offset=qbase[:, :].offset + 64 * pi * qstep + b_,
                    ap=[[qstep, 64], [512, 8], [16, 32]],
                )
                if (eps + pi) % 2 == 0:
                    nc.scalar.copy(dst, src)
                else:
                    nc.vector.tensor_copy(dst, src)
    # u-folded q variants: qlayU1 (for ac vs k), qlayU2 (for bd vs r)
    qlayU1 = qlp.tile([128, 8 * SEG], BF16, tag="qlay", name="qlayU1")
    qlayU2 = qlp.tile([128, 8 * SEG], BF16, tag="qlay", name="qlayU2")
    for g in range(8):
        sl = slice(g * SEG, (g + 1) * SEG)
        nc.vector.tensor_scalar_add(qlayU1[:, sl], qbase[:, sl], u1p[:, g:g + 1])
        nc.vector.tensor_scalar_add(qlayU2[:, sl], qbase[:, sl], u2p[:, g:g + 1])

    # ---- bd production (raw, row-packed head pairs) -> ebuf ----
    for g in range(8):
        for it in range(IT):
            m0, w_ = MMIN[it], TOT - MMIN[it]
            ebg = ebgp.tile([128, 2 * w_], BF16, tag="ebg", name=f"ebg{g}_{it}")
            for e in range(2):
                bd = ps.tile([128, w_], F32, tag="mm", name=f"bd{2 * g + e}_{it}")
                pb = 64 * e
                c0 = 0
                while c0 < w_:
                    cw = min(512, w_ - c0)
                    nc.tensor.matmul(
                        bd[:, c0:c0 + cw],
                        lhsT=qlayU2[pb:pb + 64,
                                    g * SEG + it * 128:g * SEG + (it + 1) * 128],
                        rhs=rlay[pb:pb + 64,
                                 g * TOT + m0 + c0:g * TOT + m0 + c0 + cw],
                        start=True,
                        stop=True,
                    )
                    c0 += cw
                nc.scalar.copy(ebg[:, e * w_:(e + 1) * w_], bd)
            dst = bass.AP(
                tensor=ebuf,
                offset=it * 128 * H * TOT + (2 * g) * TOT + m0,
                ap=[[H * TOT, 128], [TOT, 2], [1, w_]],
            )
            nc.sync.dma_start(dst, ebg.rearrange("p (e w) -> p e w", e=2))

    # ---- k projection -> klay2 (head-pair stacked) ----
    # klay2[64*pi + d, g*1024 + 16a + b] = Kpsum[b-chunk][64(b%2)+d, 128g+64pi+a]
    klay = lay32.tile([128, 8 * TOT], BF16, tag="lay32", name="klay")
    kst = klay[:, :].ap[0][0]
    koff = klay[:, :].offset
    wks = stream_w("wk", 1)
    for dt in range(8):
        acc = ps.tile([128, TOT], F32, tag="mm", name=f"kmm{dt}")
        for c0 in range(0, TOT, 512):
            for mt in range(8):
                nc.tensor.matmul(
                    acc[:, c0:c0 + 512],
                    lhsT=wks[mt][:, dt * 128:(dt + 1) * 128],
                    rhs=hts[mt][:, c0:c0 + 512],
                    start=(mt == 0),
                    stop=(mt == 7),
                )
        astep = acc[:, :].ap[0][0]
        for eps in range(2):
            b_ = 2 * dt + eps
            for pi in range(2):
                src = bass.AP(
                    tensor=acc[:, :].tensor,
                    offset=acc[:, :].offset + eps * 64 * astep + 64 * pi,
                    ap=[[astep, 64], [128, 8], [1, 64]],
                )
                dst = bass.AP(
                    tensor=klay[:, :].tensor,
                    offset=koff + 64 * pi * kst + 64 * b_,
                    ap=[[kst, 64], [1024, 8], [1, 64]],
                )
                if (eps + pi) % 2 == 0:
                    nc.scalar.copy(dst, src)
                else:
                    nc.vector.tensor_copy(dst, src)

    # ---- v projection per 4-head group -> vtg -> PE-transpose -> vls ----
    vls = [
        vlp.tile([128, H * 64], BF16, tag="vl", name=f"vl{jb}") for jb in range(8)
    ]
    wvs = stream_w("wv", 0)
    for half in range(2):
        vtgs = [
            vtgp.tile([64, 4 * TOT], BF16, tag="vtg", name=f"vtg{2 * half + gg}")
            for gg in range(2)
        ]
        for dt in range(8):
            acc = ps.tile([128, 512], F32, tag="mm", name=f"vmm{half}_{dt}")
            for mt in range(8):
                nc.tensor.matmul(
                    acc,
                    lhsT=wvs[mt][:, dt * 128:(dt + 1) * 128],
                    rhs=hts[mt][:, 512 * half:512 * (half + 1)],
                    start=(mt == 0),
                    stop=(mt == 7),
                )
            astep = acc[:, :].ap[0][0]
            for eps in range(2):
                b_ = 2 * dt + eps
                for gg in range(2):
                    vtg = vtgs[gg]
                    src = bass.AP(
                        tensor=acc[:, :].tensor,
                        offset=acc[:, :].offset + eps * 64 * astep + 256 * gg,
                        ap=[[astep, 64], [64, 4], [1, 64]],
                    )
                    dst = bass.AP(
                        tensor=vtg[:, :].tensor,
                        offset=vtg[:, :].offset + b_,
                        ap=[[vtg[:, :].ap[0][0], 64], [1024, 4], [16, 64]],
                    )
                    if (dt + eps + gg) % 2 == 0:
                        nc.scalar.copy(dst, src)
                    else:
                        nc.vector.tensor_copy(dst, src)
        for gg in range(2):
            g = 2 * half + gg
            vtg = vtgs[gg]
            for hh in range(4):
                h = 4 * g + hh
                for jb in range(8):
                    tp = pst.tile([128, 64], BF16, tag="tp", name=f"vt{h}_{jb}")
                    nc.tensor.transpose(
                        tp, vtg[0:64, hh * TOT + jb * 128:hh * TOT + (jb + 1) * 128],
                        ident[0:64, 0:64],
                    )
                    nc.scalar.copy(vls[jb][:, h * 64:(h + 1) * 64], tp)

    # ---- scores / softmax / p@v ----
    attTh = qbp.tile([64, H * SEG], BF16, tag="qb", name="attTh")
    for it in range(IT):
        jm = JMAX[it]
        nblk = jm // 128
        i0 = it * 128
        for hp in range(8):
            ebs = ebsp.tile([128, 2 * jm], BF16, tag="ebs", name=f"ebs{it}_{hp}")
            src = bass.AP(
                tensor=ebuf,
                offset=i0 * H * TOT + (2 * hp) * TOT + (511 - i0),
                ap=[[H * TOT - 1, 128], [TOT, 2], [1, jm]],
            )
            nc.scalar.dma_start(ebs.rearrange("p (e w) -> p e w", e=2), src)
            pts_pair = []
            for e in range(2):
                h = 2 * hp + e
                # additive triangular corner mask (covers the circulant spill)
                nc.gpsimd.tensor_add(
                    ebs[:, e * jm + jm - 128:(e + 1) * jm],
                    ebs[:, e * jm + jm - 128:(e + 1) * jm],
                    triB,
                )
                acps = ps.tile([128, jm], F32, tag="mm", name=f"ac{it}_{h}")
                pb = 64 * e
                kst_ = klay[:, :].ap[0][0]
                c0 = 0
                while c0 < jm:
                    cw = min(512, jm - c0)
                    rhs = bass.AP(
                        tensor=klay[:, :].tensor,
                        offset=klay[:, :].offset + pb * kst_ + hp * TOT + c0 // 16,
                        ap=[[kst_, 64], [1, cw // 16], [64, 16]],
                    )
                    nc.tensor.matmul(
                        acps[:, c0:c0 + cw],
                        lhsT=qlayU1[pb:pb + 64,
                                    hp * SEG + i0:hp * SEG + i0 + 128],
                        rhs=rhs,
                        start=True,
                        stop=True,
                    )
                    c0 += cw
                s_sb = workp.tile([128, jm], BF16, tag="wk", name=f"s{it}_{h}")
                nc.vector.tensor_tensor(
                    out=s_sb, in0=acps, in1=ebs[:, e * jm:(e + 1) * jm], op=OP.add
                )
                pexp = workp.tile([128, jm], BF16, tag="wk", name=f"p{it}_{h}")
                sums = small.tile([128, 1], F32, tag="sums", name=f"sm{it}_{h}")
                nc.scalar.activation(pexp, s_sb, AF.Exp, scale=0.125, accum_out=sums)
                rec = small.tile([128, 1], F32, tag="rec", name=f"rc{it}_{h}")
                nc.vector.reciprocal(rec, sums)
                alph = small.tile([128, 1], F32, tag="alph", name=f"al{it}_{h}")
                nc.vector.tensor_mul(alph, rec, masks[:, it:it + 1])
                nc.vector.tensor_scalar_mul(pexp, pexp, alph)
                pts = ptsp.tile([128, jm], BF16, tag="pts", name=f"pt{it}_{h}")
                for jb in range(nblk):
                    tp = pst.tile([128, 128], BF16, tag="tp", name=f"tp{it}_{h}_{jb}")
                    nc.tensor.transpose(tp, pexp[:, jb * 128:(jb + 1) * 128], ident)
                    if jb % 2 == 0:
                        nc.vector.tensor_copy(pts[:, jb * 128:(jb + 1) * 128], tp)
                    else:
                        nc.scalar.copy(pts[:, jb * 128:(jb + 1) * 128], tp)
                pts_pair.append(pts)
            # packed p@v: two heads as PE column-tiles into one PSUM tile
            pv = psv.tile([128, 128], F32, tag="pv", name=f"pv{it}_{hp}")
            for jb in range(nblk):
                for e in range(2):
                    h = 2 * hp + e
                    nc.tensor.matmul(
                        pv[64 * e:64 * (e + 1), :],
                        lhsT=vls[jb][:, 64 * h:64 * h + 64],
                        rhs=pts_pair[e][:, jb * 128:(jb + 1) * 128],
                        start=(jb == 0),
                        stop=(jb == nblk - 1),
                    )
            for e in range(2):
                h = 2 * hp + e
                nc.scalar.copy(
                    attTh[:, h * SEG + i0:h * SEG + i0 + 128],
                    pv[64 * e:64 * (e + 1), :],
                )

    # ---- att DRAM hop: attP[dd, cc*512+32h+rr] = attTh[dd, h*512+16rr+cc] ----
    attP = vtgp.tile([64, H * SEG], BF16, tag="vtg", name="attP")
    ao = attTh[:, :].offset
    astep = attTh[:, :].ap[0][0]
    src = bass.AP(
        tensor=attTh[:, :].tensor, offset=ao,
        ap=[[astep, 64], [1, 16], [512, 16], [16, 32]],
    )
    po = attP[:, :].offset
    pstep = attP[:, :].ap[0][0]
    dst = bass.AP(
        tensor=attP[:, :].tensor, offset=po,
        ap=[[pstep, 64], [512, 16], [32, 16], [1, 32]],
    )
    nc.vector.tensor_copy(dst, src)
    nc.sync.dma_start(
        bass.AP(tensor=t["attd"], offset=0, ap=[[H * SEG, 64], [1, H * SEG]]),
        attP,
    )
    atts = []
    for a in range(8):
        at = attp.tile([128, SEG], BF16, tag="att", name=f"att{a}")
        for ccp in range(2):
            src = bass.AP(
                tensor=t["attd"],
                offset=(2 * a + ccp) * 512,
                ap=[[H * SEG, 64], [1, 512]],
            )
            eng = nc.scalar if (a + ccp) % 2 == 0 else nc.sync
            eng.dma_start(at[ccp * 64:(ccp + 1) * 64, :], src)
        atts.append(at)

    # ---- mlp + residual + layernorm ----
    mlps = stream_w("mlpw", 1)
    xs = []
    for it in range(IT):
        x = xp.tile([128, MODEL], F32, tag="x", name=f"x{it}")
        eng = nc.sync if it % 2 == 0 else nc.scalar
        eng.dma_start(x, t["x_sm"][it * 128:(it + 1) * 128, :])
        xs.append(x)
    for it in range(IT):
        acc = ps.tile([128, MODEL], F32, tag="mm", name=f"mlp{it}")
        for half in range(2):
            for dt in range(8):
                nc.tensor.matmul(
                    acc[:, half * 512:(half + 1) * 512],
                    lhsT=atts[dt][:, it * 128:(it + 1) * 128],
                    rhs=mlps[dt][:, half * 512:(half + 1) * 512],
                    start=(dt == 0),
                    stop=(dt == 7),
                )
        y = yp.tile([128, MODEL], F32, tag="y", name=f"y{it}", bufs=1)
        ysum = small.tile([128, 1], F32, tag="ysum", name=f"ys{it}")
        nc.vector.scalar_tensor_tensor(
            out=y, in0=acc, scalar=1.0, in1=xs[it],
            op0=OP.mult, op1=OP.add, accum_out=ysum,
        )
        sq = ps.tile([128, MODEL], F32, tag="mm", name=f"sq{it}")
        ysq = small.tile([128, 1], F32, tag="ysq", name=f"yq{it}")
        nc.scalar.activation(sq, y, AF.Square, accum_out=ysq)
        mu = small.tile([128, 1], F32, tag="mu", name=f"mu{it}")
        nc.scalar.mul(mu, ysum, 1.0 / MODEL)
        msq = small.tile([128, 1], F32, tag="msq", name=f"mq{it}")
        nc.scalar.mul(msq, ysq, 1.0 / MODEL)
        mu2 = small.tile([128, 1], F32, tag="mu2", name=f"m2{it}")
        nc.vector.tensor_mul(mu2, mu, mu)
        var = small.tile([128, 1], F32, tag="var", name=f"va{it}")
        nc.vector.tensor_tensor(out=var, in0=msq, in1=mu2, op=OP.subtract)
        std = small.tile([128, 1], F32, tag="std", name=f"sd{it}")
        nc.scalar.activation(std, var, AF.Sqrt, bias=epsb)
        rstd = small.tile([128, 1], F32, tag="rstd", name=f"rs{it}")
        nc.vector.reciprocal(rstd, std)
        o = yp.tile([128, MODEL], F32, tag="o", name=f"o{it}", bufs=1)
        nc.vector.tensor_scalar(
            out=o, in0=y, scalar1=mu, scalar2=rstd,
            op0=OP.subtract, op1=OP.mult,
        )
        nc.vector.tensor_mul(o, o, gam)
        nc.vector.tensor_add(o, o, bet)
        nc.sync.dma_start(t["yout"][it * 128:(it + 1) * 128, :], o)

    for p_ in reversed(ctxs):
        p_.__exit__(None, None, None)


def _split_ctrl_waits(nc, maxw=1):
    """The container's walrus rejects instructions carrying more than 2 sem
    waits ("Too many sync wait commands"). Move excess waits onto preceding
    same-engine NoOps (engines execute their stream in order, so the waits
    still complete before the instruction issues)."""
    n = 0
    for bb in nc.main_func.blocks:
        changed = False
        out = []
        for ins in bb.instructions:
            lim = maxw
            si = ins.sync_info
            if si is not None and si.on_wait and len(si.on_wait) > lim:
                waits = list(si.on_wait)
                while len(waits) > lim:
                    chunk, waits = waits[:lim], waits[lim:]
                    nop = mybir.InstNoOp(
                        name=f"I-wsplit{n}",
                        engine=ins.engine,
                        sync_info=mybir.SyncInfo(on_wait=list(chunk), on_update=[]),
                    )
                    n += 1
                    out.append(nop)
                si.on_wait = waits
                changed = True
            out.append(ins)
        if changed:
            bb.instructions = out


@functools.lru_cache(maxsize=1)
def _build():
    nc = bass.Bass()
    t = {}
    t["hT"] = nc.dram_tensor("hT", [MODEL, TOT], BF16, kind="ExternalInput")
    t["x_sm"] = nc.dram_tensor("x_sm", [SEG, MODEL], F32, kind="ExternalInput")
    for w in ("wq", "wk", "wv"):
        t[w] = nc.dram_tensor(w, [MODEL, HD], BF16, kind="ExternalInput")
    t["mlpw"] = nc.dram_tensor("mlpw", [HD, MODEL], BF16, kind="ExternalInput")
    t["rlay"] = nc.dram_tensor("rlay", [128, 8 * TOT], BF16, kind="ExternalInput")
    t["u1p"] = nc.dram_tensor("u1p", [128, 8], F32, kind="ExternalInput")
    t["u2p"] = nc.dram_tensor("u2p", [128, 8], F32, kind="ExternalInput")
    t["maskc"] = nc.dram_tensor("maskc", [128, 4], F32, kind="ExternalInput")
    t["gam"] = nc.dram_tensor("gam", [1, MODEL], BF16, kind="ExternalInput")
    t["bet"] = nc.dram_tensor("bet", [1, MODEL], BF16, kind="ExternalInput")
    t["trim"] = nc.dram_tensor("trim", [128, 128], BF16, kind="ExternalInput")
    t["idm"] = nc.dram_tensor("idm", [128, 128], BF16, kind="ExternalInput")
    t["ebuf"] = nc.dram_tensor("ebuf", [SEG, H, TOT], BF16)
    t["attd"] = nc.dram_tensor("attd", [64, H * SEG], BF16)
    t["yout"] = nc.dram_tensor("yout", [SEG, MODEL], F32, kind="ExternalOutput")

    with tile.TileContext(nc) as tc:
        _emit(tc, t)
    _split_ctrl_waits(nc)
    return nc


def _host_inputs(inputs):
    x = np.asarray(inputs["x"], np.float32)
    mem = np.asarray(inputs["mem"], np.float32)
    att_mask = np.asarray(inputs["att_mask"], np.float32)
    u1 = np.asarray(inputs["u1"], np.float32).reshape(H, D)
    u2 = np.asarray(inputs["u2"], np.float32).reshape(H, D)
    R = np.asarray(inputs["R"], np.float32)

    h = np.concatenate([mem, x], axis=1)  # [B, TOT, MODEL]

    # host r projection + head-pair-stacked lay layout
    RW = R[-TOT:] @ np.asarray(inputs["w_r"], np.float32)           # [TOT, HD]
    rl64 = RW.reshape(16, 64, 16, 64).transpose(3, 0, 1, 2).reshape(64, H, TOT)
    rlay = np.zeros((128, 8 * TOT), np.float32)
    rlay[0:64] = rl64[:, 0::2].reshape(64, 8 * TOT)
    rlay[64:128] = rl64[:, 1::2].reshape(64, 8 * TOT)
    u1p = np.zeros((128, 8), np.float32)
    u1p[0:64] = u1.T[:, 0::2]
    u1p[64:128] = u1.T[:, 1::2]
    u2p = np.zeros((128, 8), np.float32)
    u2p[0:64] = u2.T[:, 0::2]
    u2p[64:128] = u2.T[:, 1::2]

    trim = np.where(
        np.tril(np.ones((128, 128), np.float32)) > 0, 0.0, NEG
    ).astype(bf16_np)

    shared = {
        "wq": np.asarray(inputs["w_q"], np.float32).astype(bf16_np),
        "wk": np.asarray(inputs["w_k"], np.float32).astype(bf16_np),
        "wv": np.asarray(inputs["w_v"], np.float32).astype(bf16_np),
        "mlpw": np.asarray(inputs["mlp_w"], np.float32).astype(bf16_np),
        "rlay": rlay.astype(bf16_np),
        "u1p": u1p,
        "u2p": u2p,
        "gam": np.asarray(inputs["ln_gamma"], np.float32).reshape(1, MODEL).astype(bf16_np),
        "bet": np.asarray(inputs["ln_beta"], np.float32).reshape(1, MODEL).astype(bf16_np),
        "trim": trim,
        "idm": np.eye(128, dtype=np.float32).astype(bf16_np),
    }
    in_maps = []
    for b in range(NCORES):
        m = dict(shared)
        m["hT"] = np.ascontiguousarray(h[b].T).astype(bf16_np)
        m["x_sm"] = np.ascontiguousarray(x[b])
        m["maskc"] = np.ascontiguousarray(att_mask[b].reshape(4, 128).T)
        in_maps.append(m)
    return in_maps


def kernel(**inputs) -> np.ndarray:
    from concourse.bass_utils import run_bass_kernel_spmd

    nc = _build()
    in_maps = _host_inputs(inputs)
    res = run_bass_kernel_spmd(nc, in_maps, list(range(NCORES)))
    out = np.stack([np.asarray(res.results[b]["yout"]) for b in range(NCORES)])
    return out.astype(np.float32)


# revision 45
# speedup vs baseline: 1.0889x; 1.0758x over previous
"""Trainium2 Bass kernel for Transformer-XL style relative-position MHA.

Problem shapes (hardcoded): B=8, SEG=512, MEM=512, MODEL=1024, H=16, D=64.
Sharding: pure data-parallel over batch -> core b computes batch element b.

Head split quirk (torch .view flat reinterpret): for a [T, H*D] projection P,
head h's matrix is P_h[m, d] = P[64h + m//16, 64*(m%16) + d] (T=1024; for the
[512, H*D] q projection the row term is 32h + i//16).

Per-head layouts ("lay": [d (+ u-row), h*T + m]) are built with ACT/DVE engine
copies straight out of the projection PSUM chunks: psum chunk dt holds output
channels hd in [128dt, 128dt+128) = 64b+d for b in {2dt, 2dt+1}; partition
half eps gives b = 2dt+eps, so two strided engine copies per chunk land the
data in lay order. (Engines handle arbitrary-stride APs at full rate; DMA
descriptor scatter for the same transform ran at ~1 GB/s.)

The r projection (R[-T:] @ w_r) and its lay layout + u2.r row are batch
independent and precomputed on the host (rlay input).

Scores: ac = (q + ones*u1.k-row) matmuls as before; bd is computed RAW
(no exp) into a [S, H, T] bf16 DRAM buffer and read back with the skewed
(circulant) access pattern; masking is additive (-30000 on the triangular
corner, which exactly covers the out-of-range spill), then ONE
exp((ac+bd)/8) with accum_out row sums. Softmax normalization (x att_mask)
is folded into the p@v PSUM evacuation via a PE-transposed sums row and a
partition-broadcast multiply.
"""

import functools
import sys

import numpy as np

sys.path.insert(0, "/opt/trn_rl_repo")

import ml_dtypes  # noqa: E402

import concourse.bass as bass  # noqa: E402
import concourse.mybir as mybir  # noqa: E402
import concourse.tile as tile  # noqa: E402

B, SEG, MEM, MODEL, H, D = 8, 512, 512, 1024, 16, 64
TOT = SEG + MEM
HD = H * D
NCORES = 8
IT = SEG // 128                # 4 row tiles of 128 queries
JMAX = [640, 768, 896, 1024]   # per row-tile: columns beyond are fully masked
MMIN = [384, 256, 128, 0]      # per row-tile: smallest rel index m read
NEG = -30000.0                 # additive mask (exp(NEG/8) == 0 in fp)

F32 = mybir.dt.float32
BF16 = mybir.dt.bfloat16
AF = mybir.ActivationFunctionType
OP = mybir.AluOpType

bf16_np = ml_dtypes.bfloat16


def _emit(tc, t):
    nc = tc.nc
    ctxs = []

    def pool(name, bufs, space="SBUF"):
        p = tc.tile_pool(name=name, bufs=bufs, space=space)
        ctxs.append(p)
        return p.__enter__()

    csts = pool("csts", 1)
    lay32 = pool("lay32", 2)      # rlay2 + klay2 [128, 8192] bf16 (16KB/part each)
    qlp = pool("qlp", 2)          # qlayU1 / qlayU2 [128, 4096] bf16
    qbp = pool("qbp", 1)          # qbase [128,4096] then attTh [64,8192] bf16
    vtgp = pool("vtgp", 2)        # vtg group [64, 4*TOT] bf16
    vlp = pool("vlp", 8)          # vlay [128, 16*64] bf16 x 8 j-tiles
    hts_p = pool("htsp", 8)       # hT bf16 [128, TOT]
    wpool = pool("wpool", 8)      # streaming weights [128,1024] bf16
    xp = pool("xp", 2)            # x residual f32 [128, 1024]
    ebgp = pool("ebgp", 2)        # bd pair evac [128, 2*1024] bf16
    ebsp = pool("ebsp", 3)        # skewed pair read [128, 2*1024] bf16
    workp = pool("workp", 3)      # s / pexp tiles [128, 1024] bf16
    ptsp = pool("ptsp", 2)        # pT [128, 1024] bf16
    attp = pool("attp", 8)        # attT [128,512] bf16
    yp = pool("yp", 2)            # mlp y/o f32
    small = pool("small", 3)
    ps = pool("ps", 2, space="PSUM")      # [128,1024] f32 (2 banks)
    pst = pool("pst", 3, space="PSUM")    # transposes [128,128] / [1,512]
    psv = pool("psv", 1, space="PSUM")    # pv accum [128,128] f32

    # ---- constants ----
    ident = csts.tile([128, 128], BF16, tag="ident")
    nc.scalar.dma_start(ident, t["idm"][:, :])
    triB = csts.tile([128, 128], BF16, tag="triB")
    nc.scalar.dma_start(triB, t["trim"][:, :])
    u1p = csts.tile([128, 8], F32, tag="u1p")
    nc.scalar.dma_start(u1p, t["u1p"][:, :])
    u2p = csts.tile([128, 8], F32, tag="u2p")
    nc.scalar.dma_start(u2p, t["u2p"][:, :])
    masks = csts.tile([128, 4], F32, tag="masks")
    nc.scalar.dma_start(masks, t["maskc"][:, :])
    epsb = csts.tile([128, 1], F32, tag="epsb")
    nc.vector.memset(epsb, 1e-5)
    gam = csts.tile([128, MODEL], BF16, tag="gam")
    nc.gpsimd.dma_start(gam, bass.AP(tensor=t["gam"], offset=0, ap=[[0, 128], [1, MODEL]]))
    bet = csts.tile([128, MODEL], BF16, tag="bet")
    nc.gpsimd.dma_start(bet, bass.AP(tensor=t["bet"], offset=0, ap=[[0, 128], [1, MODEL]]))

    ebuf = t["ebuf"]

    # ---- zero strip: rows [0,384) x all heads x m in [0,128) of ebuf ----
    zs = csts.tile([128, 4 * 128], BF16, tag="zs")
    nc.vector.memset(zs, 0.0)
    for blk in range(3):
        for hb in range(4):
            dst = bass.AP(
                tensor=ebuf,
                offset=blk * 128 * H * TOT + hb * 4 * TOT,
                ap=[[H * TOT, 128], [TOT, 4], [1, 128]],
            )
            nc.gpsimd.dma_start(dst, zs.rearrange("p (h m) -> p h m", h=4))

    # ---- rlay2 (host precomputed, head-pair stacked) ----
    rlay = lay32.tile([128, 8 * TOT], BF16, tag="lay32", name="rlay")
    nc.scalar.dma_start(rlay, t["rlay"][:, :])

    # ---- load hT (bf16) ----
    hts = []
    for mt in range(8):
        ht = hts_p.tile([128, TOT], BF16, tag="ht", name=f"ht{mt}")
        eng = nc.sync if mt % 2 == 0 else nc.scalar
        eng.dma_start(ht, t["hT"][mt * 128:(mt + 1) * 128, :])
        hts.append(ht)

    def stream_w(key, eng_sel=0):
        ws = []
        for mt in range(8):
            w = wpool.tile([128, HD], BF16, tag="w", name=f"{key}{mt}")
            eng = nc.sync if (mt + eng_sel) % 2 == 0 else nc.scalar
            eng.dma_start(w, t[key][mt * 128:(mt + 1) * 128, :])
            ws.append(w)
        return ws

    # ---- q projection -> qbase (head-pair stacked) via engine copies ----
    # qbase[64*pi + d, g*512 + 16a + b] = Qpsum[b-chunk][64(b%2)+d, 64g+32pi+a]
    qbase = qbp.tile([128, 8 * SEG], BF16, tag="qb")
    wqs = stream_w("wq", 0)
    for dt in range(8):
        acc = ps.tile([128, SEG], F32, tag="mm", name=f"qmm{dt}")
        for mt in range(8):
            nc.tensor.matmul(
                acc,
                lhsT=wqs[mt][:, dt * 128:(dt + 1) * 128],
                rhs=hts[mt][:, SEG:],
                start=(mt == 0),
                stop=(mt == 7),
            )
        qstep = qbase[:, :].ap[0][0]
        astep = acc[:, :].ap[0][0]
        for eps in range(2):
            b_ = 2 * dt + eps
            for pi in range(2):
                src = bass.AP(
                    tensor=acc[:, :].tensor,
                    offset=acc[:, :].offset + eps * 64 * astep + 32 * pi,
                    ap=[[astep, 64], [64, 8], [1, 32]],
                )
                dst = bass.AP(
                    tensor=qbase[:, :].tensor,
                    offset=qbase[:, :].offset + 64 * pi * qstep + b_,
                    ap=[[qstep, 64], [512, 8], [16, 32]],
                )
                if (eps + pi) % 2 == 0:
                    nc.scalar.copy(dst, src)
                else:
                    nc.vector.tensor_copy(dst, src)
    # u-folded q variants: qlayU1 (for ac vs k), qlayU2 (for bd vs r)
    qlayU1 = qlp.tile([128, 8 * SEG], BF16, tag="qlay", name="qlayU1")
    qlayU2 = qlp.tile([128, 8 * SEG], BF16, tag="qlay", name="qlayU2")
    for g in range(8):
        sl = slice(g * SEG, (g + 1) * SEG)
        nc.vector.tensor_scalar_add(qlayU1[:, sl], qbase[:, sl], u1p[:, g:g + 1])
        nc.vector.tensor_scalar_add(qlayU2[:, sl], qbase[:, sl], u2p[:, g:g + 1])

    # ---- bd production (raw, row-packed head pairs) -> ebuf ----
    for g in range(8):
        for it in range(IT):
            m0, w_ = MMIN[it], TOT - MMIN[it]
            ebg = ebgp.tile([128, 2 * w_], BF16, tag="ebg", name=f"ebg{g}_{it}")
            for e in range(2):
                bd = ps.tile([128, w_], F32, tag="mm", name=f"bd{2 * g + e}_{it}")
                pb = 64 * e
                c0 = 0
                while c0 < w_:
                    cw = min(512, w_ - c0)
                    nc.tensor.matmul(
                        bd[:, c0:c0 + cw],
                        lhsT=qlayU2[pb:pb + 64,
                                    g * SEG + it * 128:g * SEG + (it + 1) * 128],
                        rhs=rlay[pb:pb + 64,
                                 g * TOT + m0 + c0:g * TOT + m0 + c0 + cw],
                        start=True,
                        stop=True,
                    )
                    c0 += cw
                nc.scalar.copy(ebg[:, e * w_:(e + 1) * w_], bd)
            dst = bass.AP(
                tensor=ebuf,
                offset=it * 128 * H * TOT + (2 * g) * TOT + m0,
                ap=[[H * TOT, 128], [TOT, 2], [1, w_]],
            )
            nc.sync.dma_start(dst, ebg.rearrange("p (e w) -> p e w", e=2))

    # ---- k projection -> klay2 (head-pair stacked) ----
    # klay2[64*pi + d, g*1024 + 16a + b] = Kpsum[b-chunk][64(b%2)+d, 128g+64pi+a]
    klay = lay32.tile([128, 8 * TOT], BF16, tag="lay32", name="klay")
    kst = klay[:, :].ap[0][0]
    koff = klay[:, :].offset
    wks = stream_w("wk", 1)
    for dt in range(8):
        acc = ps.tile([128, TOT], F32, tag="mm", name=f"kmm{dt}")
        for c0 in range(0, TOT, 512):
            for mt in range(8):
                nc.tensor.matmul(
                    acc[:, c0:c0 + 512],
                    lhsT=wks[mt][:, dt * 128:(dt + 1) * 128],
                    rhs=hts[mt][:, c0:c0 + 512],
                    start=(mt == 0),
                    stop=(mt == 7),
                )
        astep = acc[:, :].ap[0][0]
        for eps in range(2):
            b_ = 2 * dt + eps
            for pi in range(2):
                src = bass.AP(
                    tensor=acc[:, :].tensor,
                    offset=acc[:, :].offset + eps * 64 * astep + 64 * pi,
                    ap=[[astep, 64], [128, 8], [1, 64]],
                )
                dst = bass.AP(
                    tensor=klay[:, :].tensor,
                    offset=koff + 64 * pi * kst + 64 * b_,
                    ap=[[kst, 64], [1024, 8], [1, 64]],
                )
                if (eps + pi) % 2 == 0:
                    nc.scalar.copy(dst, src)
                else:
                    nc.vector.tensor_copy(dst, src)

    # ---- v projection per 4-head group -> vtg -> PE-transpose -> vls ----
    vls = [
        vlp.tile([128, H * 64], BF16, tag="vl", name=f"vl{jb}") for jb in range(8)
    ]
    wvs = stream_w("wv", 0)
    for half in range(2):
        vtgs = [
            vtgp.tile([64, 4 * TOT], BF16, tag="vtg", name=f"vtg{2 * half + gg}")
            for gg in range(2)
        ]
        for dt in range(8):
            acc = ps.tile([128, 512], F32, tag="mm", name=f"vmm{half}_{dt}")
            for mt in range(8):
                nc.tensor.matmul(
                    acc,
                    lhsT=wvs[mt][:, dt * 128:(dt + 1) * 128],
                    rhs=hts[mt][:, 512 * half:512 * (half + 1)],
                    start=(mt == 0),
                    stop=(mt == 7),
                )
            astep = acc[:, :].ap[0][0]
            for eps in range(2):
                b_ = 2 * dt + eps
                for gg in range(2):
                    vtg = vtgs[gg]
                    src = bass.AP(
                        tensor=acc[:, :].tensor,
                        offset=acc[:, :].offset + eps * 64 * astep + 256 * gg,
                        ap=[[astep, 64], [64, 4], [1, 64]],
                    )
                    dst = bass.AP(
                        tensor=vtg[:, :].tensor,
                        offset=vtg[:, :].offset + b_,
                        ap=[[vtg[:, :].ap[0][0], 64], [1024, 4], [16, 64]],
                    )
                    if (dt + eps + gg) % 2 == 0:
                        nc.scalar.copy(dst, src)
                    else:
                        nc.vector.tensor_copy(dst, src)
        for gg in range(2):
            g = 2 * half + gg
            vtg = vtgs[gg]
            for hh in range(4):
                h = 4 * g + hh
                for jb in range(8):
                    tp = pst.tile([128, 64], BF16, tag="tp", name=f"vt{h}_{jb}")
                    nc.tensor.transpose(
                        tp, vtg[0:64, hh * TOT + jb * 128:hh * TOT + (jb + 1) * 128],
                        ident[0:64, 0:64],
                    )
                    nc.scalar.copy(vls[jb][:, h * 64:(h + 1) * 64], tp)

    # ---- scores / softmax / p@v ----
    attTh = qbp.tile([64, H * SEG], BF16, tag="qb", name="attTh")
    for it in range(IT):
        jm = JMAX[it]
        nblk = jm // 128
        i0 = it * 128
        for hp in range(8):
            ebs = ebsp.tile([128, 2 * jm], BF16, tag="ebs", name=f"ebs{it}_{hp}")
            src = bass.AP(
                tensor=ebuf,
                offset=i0 * H * TOT + (2 * hp) * TOT + (511 - i0),
                ap=[[H * TOT - 1, 128], [TOT, 2], [1, jm]],
            )
            nc.scalar.dma_start(ebs.rearrange("p (e w) -> p e w", e=2), src)
            pts_pair = []
            for e in range(2):
                h = 2 * hp + e
                # additive triangular corner mask (covers the circulant spill)
                nc.gpsimd.tensor_add(
                    ebs[:, e * jm + jm - 128:(e + 1) * jm],
                    ebs[:, e * jm + jm - 128:(e + 1) * jm],
                    triB,
                )
                acps = ps.tile([128, jm], F32, tag="mm", name=f"ac{it}_{h}")
                pb = 64 * e
                kst_ = klay[:, :].ap[0][0]
                c0 = 0
                while c0 < jm:
                    cw = min(512, jm - c0)
                    rhs = bass.AP(
                        tensor=klay[:, :].tensor,
                        offset=klay[:, :].offset + pb * kst_ + hp * TOT + c0 // 16,
                        ap=[[kst_, 64], [1, cw // 16], [64, 16]],
                    )
                    nc.tensor.matmul(
                        acps[:, c0:c0 + cw],
                        lhsT=qlayU1[pb:pb + 64,
                                    hp * SEG + i0:hp * SEG + i0 + 128],
                        rhs=rhs,
                        start=True,
                        stop=True,
                    )
                    c0 += cw
                s_sb = workp.tile([128, jm], BF16, tag="wk", name=f"s{it}_{h}")
                nc.vector.tensor_tensor(
                    out=s_sb, in0=acps, in1=ebs[:, e * jm:(e + 1) * jm], op=OP.add
                )
                pexp = workp.tile([128, jm], BF16, tag="wk", name=f"p{it}_{h}")
                sums = small.tile([128, 1], F32, tag="sums", name=f"sm{it}_{h}")
                nc.scalar.activation(pexp, s_sb, AF.Exp, scale=0.125, accum_out=sums)
                rec = small.tile([128, 1], F32, tag="rec", name=f"rc{it}_{h}")
                nc.vector.reciprocal(rec, sums)
                alph = small.tile([128, 1], F32, tag="alph", name=f"al{it}_{h}")
                nc.vector.tensor_mul(alph, rec, masks[:, it:it + 1])
                nc.vector.tensor_scalar_mul(pexp, pexp, alph)
                pts = ptsp.tile([128, jm], BF16, tag="pts", name=f"pt{it}_{h}")
                for jb in range(nblk):
                    tp = pst.tile([128, 128], BF16, tag="tp", name=f"tp{it}_{h}_{jb}")
                    nc.tensor.transpose(tp, pexp[:, jb * 128:(jb + 1) * 128], ident)
                    if jb % 2 == 0:
                        nc.vector.tensor_copy(pts[:, jb * 128:(jb + 1) * 128], tp)
                    else:
                        nc.scalar.copy(pts[:, jb * 128:(jb + 1) * 128], tp)
                pts_pair.append(pts)
            # packed p@v: two heads as PE column-tiles into one PSUM tile
            pv = psv.tile([128, 128], F32, tag="pv", name=f"pv{it}_{hp}")
            for jb in range(nblk):
                for e in range(2):
                    h = 2 * hp + e
                    nc.tensor.matmul(
                        pv[64 * e:64 * (e + 1), :],
                        lhsT=vls[jb][:, 64 * h:64 * h + 64],
                        rhs=pts_pair[e][:, jb * 128:(jb + 1) * 128],
                        start=(jb == 0),
                        stop=(jb == nblk - 1),
                    )
            for e in range(2):
                h = 2 * hp + e
                nc.scalar.copy(
                    attTh[:, h * SEG + i0:h * SEG + i0 + 128],
                    pv[64 * e:64 * (e + 1), :],
                )

    # ---- att DRAM hop: attP[dd, cc*512+32h+rr] = attTh[dd, h*512+16rr+cc] ----
    attP = vtgp.tile([64, H * SEG], BF16, tag="vtg", name="attP")
    ao = attTh[:, :].offset
    astep = attTh[:, :].ap[0][0]
    src = bass.AP(
        tensor=attTh[:, :].tensor, offset=ao,
        ap=[[astep, 64], [1, 16], [512, 16], [16, 32]],
    )
    po = attP[:, :].offset
    pstep = attP[:, :].ap[0][0]
    dst = bass.AP(
        tensor=attP[:, :].tensor, offset=po,
        ap=[[pstep, 64], [512, 16], [32, 16], [1, 32]],
    )
    nc.vector.tensor_copy(dst, src)
    nc.sync.dma_start(
        bass.AP(tensor=t["attd"], offset=0, ap=[[H * SEG, 64], [1, H * SEG]]),
        attP,
    )
    atts = []
    for a in range(8):
        at = attp.tile([128, SEG], BF16, tag="att", name=f"att{a}")
        for ccp in range(2):
            src = bass.AP(
                tensor=t["attd"],
                offset=(2 * a + ccp) * 512,
                ap=[[H * SEG, 64], [1, 512]],
            )
            eng = nc.scalar if (a + ccp) % 2 == 0 else nc.sync
            eng.dma_start(at[ccp * 64:(ccp + 1) * 64, :], src)
        atts.append(at)

    # ---- mlp + residual + layernorm ----
    mlps = stream_w("mlpw", 1)
    xs = []
    for it in range(IT):
        x = xp.tile([128, MODEL], F32, tag="x", name=f"x{it}")
        eng = nc.sync if it % 2 == 0 else nc.scalar
        eng.dma_start(x, t["x_sm"][it * 128:(it + 1) * 128, :])
        xs.append(x)
    for it in range(IT):
        acc = ps.tile([128, MODEL], F32, tag="mm", name=f"mlp{it}")
        for half in range(2):
            for dt in range(8):
                nc.tensor.matmul(
                    acc[:, half * 512:(half + 1) * 512],
                    lhsT=atts[dt][:, it * 128:(it + 1) * 128],
                    rhs=mlps[dt][:, half * 512:(half + 1) * 512],
                    start=(dt == 0),
                    stop=(dt == 7),
                )
        y = yp.tile([128, MODEL], F32, tag="y", name=f"y{it}", bufs=1)
        ysum = small.tile([128, 1], F32, tag="ysum", name=f"ys{it}")
        nc.vector.scalar_tensor_tensor(
            out=y, in0=acc, scalar=1.0, in1=xs[it],
            op0=OP.mult, op1=OP.add, accum_out=ysum,
        )
        sq = ps.tile([128, MODEL], F32, tag="mm", name=f"sq{it}")
        ysq = small.tile([128, 1], F32, tag="ysq", name=f"yq{it}")
        nc.scalar.activation(sq, y, AF.Square, accum_out=ysq)
        mu = small.tile([128, 1], F32, tag="mu", name=f"mu{it}")
        nc.scalar.mul(mu, ysum, 1.0 / MODEL)
        msq = small.tile([128, 1], F32, tag="msq", name=f"mq{it}")
        nc.scalar.mul(msq, ysq, 1.0 / MODEL)
        mu2 = small.tile([128, 1], F32, tag="mu2", name=f"m2{it}")
        nc.vector.tensor_mul(mu2, mu, mu)
        var = small.tile([128, 1], F32, tag="var", name=f"va{it}")
        nc.vector.tensor_tensor(out=var, in0=msq, in1=mu2, op=OP.subtract)
        std = small.tile([128, 1], F32, tag="std", name=f"sd{it}")
        nc.scalar.activation(std, var, AF.Sqrt, bias=epsb)
        rstd = small.tile([128, 1], F32, tag="rstd", name=f"rs{it}")
        nc.vector.reciprocal(rstd, std)
        o = yp.tile([128, MODEL], F32, tag="o", name=f"o{it}", bufs=1)
        nc.vector.tensor_scalar(
            out=o, in0=y, scalar1=mu, scalar2=rstd,
            op0=OP.subtract, op1=OP.mult,
        )
        nc.vector.tensor_mul(o, o, gam)
        nc.vector.tensor_add(o, o, bet)
        nc.sync.dma_start(t["yout"][it * 128:(it + 1) * 128, :], o)

    for p_ in reversed(ctxs):
        p_.__exit__(None, None, None)


def _split_ctrl_waits(nc, maxw=1):
    """The container's walrus rejects instructions carrying more than 2 sem
    waits ("Too many sync wait commands"). Move excess waits onto preceding
    same-engine NoOps (engines execute their stream in order, so the waits
    still complete before the instruction issues)."""
    n = 0
    for bb in nc.main_func.blocks:
        changed = False
        out = []
        for ins in bb.instructions:
            lim = maxw
            si = ins.sync_info
            if si is not None and si.on_wait and len(si.on_wait) > lim:
                waits = list(si.on_wait)
                while len(waits) > lim:
                    chunk, waits = waits[:lim], waits[lim:]
                    nop = mybir.InstNoOp(
                        name=f"I-wsplit{n}",
                        engine=ins.engine,
                        sync_info=mybir.SyncInfo(on_wait=list(chunk), on_update=[]),
                    )
                    n += 1
                    out.append(nop)
                si.on_wait = waits
                changed = True
            out.append(ins)
        if changed:
            bb.instructions = out


@functools.lru_cache(maxsize=1)
def _build():
    nc = bass.Bass()
    t = {}
    t["hT"] = nc.dram_tensor("hT", [MODEL, TOT], BF16, kind="ExternalInput")
    t["x_sm"] = nc.dram_tensor("x_sm", [SEG, MODEL], F32, kind="ExternalInput")
    for w in ("wq", "wk", "wv"):
        t[w] = nc.dram_tensor(w, [MODEL, HD], BF16, kind="ExternalInput")
    t["mlpw"] = nc.dram_tensor("mlpw", [HD, MODEL], BF16, kind="ExternalInput")
    t["rlay"] = nc.dram_tensor("rlay", [128, 8 * TOT], BF16, kind="ExternalInput")
    t["u1p"] = nc.dram_tensor("u1p", [128, 8], F32, kind="ExternalInput")
    t["u2p"] = nc.dram_tensor("u2p", [128, 8], F32, kind="ExternalInput")
    t["maskc"] = nc.dram_tensor("maskc", [128, 4], F32, kind="ExternalInput")
    t["gam"] = nc.dram_tensor("gam", [1, MODEL], BF16, kind="ExternalInput")
    t["bet"] = nc.dram_tensor("bet", [1, MODEL], BF16, kind="ExternalInput")
    t["trim"] = nc.dram_tensor("trim", [128, 128], BF16, kind="ExternalInput")
    t["idm"] = nc.dram_tensor("idm", [128, 128], BF16, kind="ExternalInput")
    t["ebuf"] = nc.dram_tensor("ebuf", [SEG, H, TOT], BF16)
    t["attd"] = nc.dram_tensor("attd", [64, H * SEG], BF16)
    t["yout"] = nc.dram_tensor("yout", [SEG, MODEL], F32, kind="ExternalOutput")

    with tile.TileContext(nc) as tc:
        _emit(tc, t)
    _split_ctrl_waits(nc)
    return nc


def _host_inputs(inputs):
    x = np.asarray(inputs["x"], np.float32)
    mem = np.asarray(inputs["mem"], np.float32)
    att_mask = np.asarray(inputs["att_mask"], np.float32)
    u1 = np.asarray(inputs["u1"], np.float32).reshape(H, D)
    u2 = np.asarray(inputs["u2"], np.float32).reshape(H, D)
    R = np.asarray(inputs["R"], np.float32)

    h = np.concatenate([mem, x], axis=1)  # [B, TOT, MODEL]

    # host r projection + head-pair-stacked lay layout
    RW = R[-TOT:] @ np.asarray(inputs["w_r"], np.float32)           # [TOT, HD]
    rl64 = RW.reshape(16, 64, 16, 64).transpose(3, 0, 1, 2).reshape(64, H, TOT)
    rlay = np.zeros((128, 8 * TOT), np.float32)
    rlay[0:64] = rl64[:, 0::2].reshape(64, 8 * TOT)
    rlay[64:128] = rl64[:, 1::2].reshape(64, 8 * TOT)
    u1p = np.zeros((128, 8), np.float32)
    u1p[0:64] = u1.T[:, 0::2]
    u1p[64:128] = u1.T[:, 1::2]
    u2p = np.zeros((128, 8), np.float32)
    u2p[0:64] = u2.T[:, 0::2]
    u2p[64:128] = u2.T[:, 1::2]

    trim = np.where(
        np.tril(np.ones((128, 128), np.float32)) > 0, 0.0, NEG
    ).astype(bf16_np)

    shared = {
        "wq": np.asarray(inputs["w_q"], np.float32).astype(bf16_np),
        "wk": np.asarray(inputs["w_k"], np.float32).astype(bf16_np),
        "wv": np.asarray(inputs["w_v"], np.float32).astype(bf16_np),
        "mlpw": np.asarray(inputs["mlp_w"], np.float32).astype(bf16_np),
        "rlay": rlay.astype(bf16_np),
        "u1p": u1p,
        "u2p": u2p,
        "gam": np.asarray(inputs["ln_gamma"], np.float32).reshape(1, MODEL).astype(bf16_np),
        "bet": np.asarray(inputs["ln_beta"], np.float32).reshape(1, MODEL).astype(bf16_np),
        "trim": trim,
        "idm": np.eye(128, dtype=np.float32).astype(bf16_np),
    }
    in_maps = []
    for b in range(NCORES):
        m = dict(shared)
        m["hT"] = np.ascontiguousarray(h[b].T).astype(bf16_np)
        m["x_sm"] = np.ascontiguousarray(x[b])
        m["maskc"] = np.ascontiguousarray(att_mask[b].reshape(4, 128).T)
        in_maps.append(m)
    return in_maps


def kernel(**inputs) -> np.ndarray:
    from concourse.bass_utils import run_bass_kernel_spmd

    nc = _build()
    in_maps = _host_inputs(inputs)
    res = run_bass_kernel_spmd(nc, in_maps, list(range(NCORES)))
    out = np.stack([np.asarray(res.results[b]["yout"]) for b in range(NCORES)])
    return out.astype(np.float32)
